# revision 1
# baseline (speedup 1.0000x reference)
"""MoE gate kernel for Trainium2 (8 NeuronCores, SPMD data-parallel over tokens).

reference:
    scores = sigmoid(x @ W.T)            # [T, E] fp32
    biased = scores + bias
    inds   = top_k(-biased, 8).indices   # 8 smallest biased, order ascending biased
    sel    = scores[inds] / sum * 2.5

Device strategy (per core, 2048 tokens):
  - x and W.T are split on the host into fp16 hi + fp16 lo residual pairs
    (22-bit mantissa coverage -> logits match the fp32 reference to ~1e-6;
    lo residuals may be fp16 subnormals, which the PE multiplies exactly).
    x is additionally pre-tiled on the host into the exact per-partition
    SBUF layout (h-block on partitions) so every DMA descriptor moves
    contiguous KBs instead of 128B strips.
  - logits = xh@wh + xh@wl + xl@wh  (3 fp16 matmuls, one PSUM accumulation)
    x is the stationary PE operand so PSUM comes out as [tokens, experts]:
    no transposes anywhere on device.
  - ACT sigmoid, DVE max/max_index for top-8 (matches jax tie-breaking),
    iota-equality scalar_tensor_tensor with accum_out to gather the selected
    original scores, reciprocal-normalize, scale by 2.5.
  - Output: one [128, NT*16] u32 buffer; per token 8 idx words + 8 fp32-bit
    sel words; host un-permutes.
"""

import sys

sys.path.insert(0, "/opt/trn_rl_repo")

import numpy as np

import concourse.bacc as bacc
import concourse.mybir as mybir
import concourse.tile as tile
from concourse import bass_utils

T, H, E, K = 16384, 4096, 256, 8
N_CORES = 8
TS = T // N_CORES          # tokens per core
TCHUNK = 128               # tokens per PE tile (PSUM partition dim)
NT = TS // TCHUNK          # token tiles per core
F = H // 128               # h-slices per partition block
ROUTED_SCALING = 2.5

f32 = mybir.dt.float32
f16 = mybir.dt.float16
u32 = mybir.dt.uint32
Alu = mybir.AluOpType
Act = mybir.ActivationFunctionType


def build_nc(nt=NT):
    """Build the SPMD Bass program for one core handling nt*TCHUNK tokens."""
    nc = bacc.Bacc("TRN2", target_bir_lowering=False, debug=False,
                   num_devices=N_CORES)

    # x pre-tiled on host: [it, p, f*TCHUNK + t] = x[it*TCHUNK + t, 32p + f]
    xth_d = nc.dram_tensor("xth", [nt, 128, F * TCHUNK], f16,
                           kind="ExternalInput")
    xtl_d = nc.dram_tensor("xtl", [nt, 128, F * TCHUNK], f16,
                           kind="ExternalInput")
    wth_d = nc.dram_tensor("wth", [H, E], f16, kind="ExternalInput")
    wtl_d = nc.dram_tensor("wtl", [H, E], f16, kind="ExternalInput")
    nbias_d = nc.dram_tensor("nbias", [128, E], f32, kind="ExternalInput")
    iota_d = nc.dram_tensor("iota", [128, E], f32, kind="ExternalInput")
    out_d = nc.dram_tensor("out", [128, nt * 2 * K], u32, kind="ExternalOutput")

    with tile.TileContext(nc) as tc:
        with (
            tc.tile_pool(name="const", bufs=1) as cpool,
            tc.tile_pool(name="xp", bufs=4) as xpool,
            tc.tile_pool(name="sc", bufs=4) as spool,
            tc.tile_pool(name="sm", bufs=4) as smpool,
            tc.tile_pool(name="ps", bufs=8, space="PSUM") as ppool,
        ):
            # weights + consts on the ACT HWDGE queue, x tiles on the SP
            # queue, so the startup loads run in parallel. Each weight chunk
            # is its own tile so the f=0 matmuls depend only on chunk 0.
            FC = F // 8
            wth_src = wth_d.ap().rearrange("(p f) e -> p f e", f=F)
            wtl_src = wtl_d.ap().rearrange("(p f) e -> p f e", f=F)
            # all wth chunks before wtl: the hh matmul phase only needs wth,
            # so the cross-term weights stream while hh matmuls already run.
            wth_c, wtl_c = [], []
            for c in range(8):
                fs = slice(c * FC, (c + 1) * FC)
                th = cpool.tile([128, FC, E], f16, tag=f"wth{c}")
                nc.scalar.dma_start(th[:], wth_src[:, fs, :])
                wth_c.append(th)
            for c in range(8):
                fs = slice(c * FC, (c + 1) * FC)
                tl = cpool.tile([128, FC, E], f16, tag=f"wtl{c}")
                nc.scalar.dma_start(tl[:], wtl_src[:, fs, :])
                wtl_c.append(tl)
            nb = cpool.tile([128, E], f32, tag="nb")
            nc.scalar.dma_start(nb[:], nbias_d.ap())
            io = cpool.tile([128, E], f32, tag="io")
            nc.scalar.dma_start(io[:], iota_d.ap())
            scrv = cpool.tile([128, E], f32, tag="scrv")
            obuf = cpool.tile([128, nt * 2 * K], u32, tag="obuf")

            FH = F // 4
            for it in range(nt):
                xh_src = xth_d.ap()[it].rearrange("p (f t) -> p f t", f=F)
                xl_src = xtl_d.ap()[it].rearrange("p (f t) -> p f t", f=F)
                xh_h, xl_h = [], []
                for c in range(4):
                    fs = slice(c * FH, (c + 1) * FH)
                    th = xpool.tile([128, FH, TCHUNK], f16, tag=f"xh{c}")
                    nc.sync.dma_start(th[:], xh_src[:, fs, :])
                    xh_h.append(th)
                for c in range(4):
                    fs = slice(c * FH, (c + 1) * FH)
                    tl = xpool.tile([128, FH, TCHUNK], f16, tag=f"xl{c}")
                    nc.sync.dma_start(tl[:], xl_src[:, fs, :])
                    xl_h.append(tl)

                # the lo parts are raw fp16 residuals (subnormals included —
                # the PE handles them exactly), so all three terms accumulate
                # into a single PSUM bank.
                acc = ppool.tile([128, E], f32, tag="acc")
                for f in range(F):
                    nc.tensor.matmul(acc[:], xh_h[f // FH][:, f % FH, :],
                                     wth_c[f // FC][:, f % FC, :],
                                     start=(f == 0), stop=False)
                for f in range(F):
                    nc.tensor.matmul(acc[:], xh_h[f // FH][:, f % FH, :],
                                     wtl_c[f // FC][:, f % FC, :],
                                     start=False, stop=False)
                for f in range(F):
                    nc.tensor.matmul(acc[:], xl_h[f // FH][:, f % FH, :],
                                     wth_c[f // FC][:, f % FC, :],
                                     start=False, stop=(f == F - 1))

                scores = spool.tile([128, E], f32, tag="scores")
                nc.scalar.activation(scores[:], acc[:], Act.Sigmoid)

                negb = spool.tile([128, E], f32, tag="negb")
                nc.vector.tensor_tensor(negb[:], nb[:], scores[:], Alu.subtract)
                m8 = smpool.tile([128, K], f32, tag="m8")
                idx = obuf[:, it * 2 * K: it * 2 * K + K]
                nc.vector.max(m8[:], negb[:])
                nc.vector.max_index(idx, m8[:], negb[:])
                idxf = smpool.tile([128, K], f32, tag="idxf")
                nc.vector.tensor_copy(idxf[:], idx)

                gath = smpool.tile([128, K], f32, tag="gath")
                for j in range(K):
                    nc.vector.scalar_tensor_tensor(
                        scrv[:], io[:], idxf[:, j:j + 1], scores[:],
                        Alu.is_equal, Alu.mult,
                        accum_out=gath[:, j:j + 1])

                ssum = smpool.tile([128, 1], f32, tag="ssum")
                nc.vector.tensor_reduce(ssum[:], gath[:],
                                        mybir.AxisListType.X, Alu.add)
                rec = smpool.tile([128, 1], f32, tag="rec")
                nc.vector.reciprocal(rec[:], ssum[:])

                nc.vector.tensor_scalar(
                    obuf[:, it * 2 * K + K: (it + 1) * 2 * K].bitcast(f32),
                    gath[:], rec[:], ROUTED_SCALING, Alu.mult, Alu.mult)

            nc.sync.dma_start(out_d.ap(), obuf[:])

    nc.compile()
    return nc


def host_prep(x, weight, e_score_correction_bias):
    """Split inputs into fp16 hi/lo pairs, pre-tile x, build per-core maps."""
    x = np.asarray(x, dtype=np.float32)
    w = np.asarray(weight, dtype=np.float32)
    b = np.asarray(e_score_correction_bias, dtype=np.float32)

    xh = x.astype(np.float16)
    xl = (x - xh.astype(np.float32)).astype(np.float16)

    def pretile(a):  # [TS, H] -> [NT, 128, F*TCHUNK]; [it,p,f,t]=a[it*128+t,32p+f]
        a = a.reshape(NT, TCHUNK, 128, F).transpose(0, 2, 3, 1)
        return np.ascontiguousarray(a).reshape(NT, 128, F * TCHUNK)

    wt = np.ascontiguousarray(w.T)     # [H, E]
    wth = wt.astype(np.float16)
    wtl = (wt - wth.astype(np.float32)).astype(np.float16)

    nbias = np.ascontiguousarray(np.broadcast_to(-b, (128, E)))
    iota = np.ascontiguousarray(
        np.broadcast_to(np.arange(E, dtype=np.float32), (128, E)))

    in_maps = []
    for c in range(N_CORES):
        sl = slice(c * TS, (c + 1) * TS)
        in_maps.append({
            "xth": pretile(xh[sl]),
            "xtl": pretile(xl[sl]),
            "wth": wth,
            "wtl": wtl,
            "nbias": nbias,
            "iota": iota,
        })
    return in_maps


def unpack(out_cores):
    """list of [128, NT*16] u32 -> (inds int32 [T, 8], sel float32 [T, 8])."""
    inds = np.empty((T, K), dtype=np.int32)
    sel = np.empty((T, K), dtype=np.float32)
    for c, o in enumerate(out_cores):
        o = o.reshape(128, NT, 2 * K).transpose(1, 0, 2)  # [it, p, 16]
        o = np.ascontiguousarray(o).reshape(TS, 2 * K)
        inds[c * TS:(c + 1) * TS] = o[:, :K].astype(np.int32)
        sel[c * TS:(c + 1) * TS] = o[:, K:].view(np.float32)
    return inds, sel


_NC_CACHE = {}


def _get_nc():
    if "nc" not in _NC_CACHE:
        _NC_CACHE["nc"] = build_nc()
    return _NC_CACHE["nc"]


def kernel(x, weight, e_score_correction_bias, _trace=False):
    in_maps = host_prep(x, weight, e_score_correction_bias)
    nc = _get_nc()
    res = bass_utils.run_bass_kernel_spmd(
        nc, in_maps, list(range(N_CORES)), trace=_trace)
    inds, sel = unpack([res.results[c]["out"] for c in range(N_CORES)])
    if _trace:
        kernel.last_results = res
    return inds, sel



# revision 5
# speedup vs baseline: 1.6589x; 1.6589x over previous
"""MoE gate kernel for Trainium2 (8 NeuronCores, SPMD data-parallel over tokens).

reference:
    scores = sigmoid(x @ W.T)            # [T, E] fp32
    biased = scores + bias
    inds   = top_k(-biased, 8).indices   # 8 smallest biased, ascending biased
    sel    = scores[inds] / sum * 2.5

Numerics: logits are computed to ~1.3e-5 abs error (vs logit std 1.28) with
2.0 fp16-matmul-equivalents of PE work instead of the naive 3:
    x  = a + b,   a = fp16(x),  b = x - a
    Wt = c + d,   c = fp16(Wt), d = Wt - c
    x@Wt ~= a@c  +  a@d + b@c      (b@d ~ 2^-22 relative, dropped)
  - main term:   (a*2^7) @ (c*2^7)            fp16 matmul,   scale 2^14
  - corrections: e4m3(a) @ e4m3(d*2^14)       fp8 DoubleRow (2x rate)
                 e4m3(b*2^11) @ e4m3(c*2^3)   fp8 DoubleRow
  All terms accumulate into ONE PSUM bank at common scale 2^14; the ACT
  sigmoid applies scale=2^-14 on read. HW fp16/fp8 matmul numerics verified
  bit-close to the numpy/ml_dtypes simulation (6/16384 tokens flip an index,
  combined rel err 0.005 << 0.02 gate).

Device flow per 128-token tile: 32 fp16 matmuls + 32 fp8 DoubleRow matmuls
(contraction 256/instr) -> sigmoid(scale) -> DVE keys = -bias - scores ->
DVE max8/max_index -> 8 idx + 8 key words per token. The host recovers the
selected original scores as s_j = -key_j - bias[idx_j] (exact algebra,
O(T*K) work) and normalizes: sel = 2.5 * s / s.sum().
"""

import sys

sys.path.insert(0, "/opt/trn_rl_repo")

import numpy as np
import ml_dtypes

import concourse.bacc as bacc
import concourse.mybir as mybir
import concourse.tile as tile
from concourse import bass_utils

T, H, E, K = 16384, 4096, 256, 8
N_CORES = 8
TS = T // N_CORES          # tokens per core
TCHUNK = 128               # tokens per PE tile (PSUM partition dim)
NT = TS // TCHUNK          # token tiles per core
F = H // 128               # h-slices per partition block
ROUTED_SCALING = 2.5
E4 = ml_dtypes.float8_e4m3

f32 = mybir.dt.float32
f16 = mybir.dt.float16
f8 = mybir.dt.float8e4
u32 = mybir.dt.uint32
Alu = mybir.AluOpType
Act = mybir.ActivationFunctionType
DR = mybir.MatmulPerfMode.DoubleRow


def build_nc(nt=NT):
    """Build the SPMD Bass program for one core handling nt*TCHUNK tokens."""
    nc = bacc.Bacc("TRN2", target_bir_lowering=False, debug=False,
                   num_devices=N_CORES)

    # pre-tiled on host: [it, p, f*TCHUNK + t] = arr[it*TCHUNK + t, 32p + f]
    a16_d = nc.dram_tensor("a16", [nt, 128, F * TCHUNK], f16,
                           kind="ExternalInput")
    # fp8 x payload: [it, p, kt, t]; kt<F -> e4m3(a), kt>=F -> e4m3(b*2^11)
    x8_d = nc.dram_tensor("x8", [nt, 128, 2 * F * TCHUNK], f8,
                          kind="ExternalInput")
    c16_d = nc.dram_tensor("c16", [H, E], f16, kind="ExternalInput")
    # fp8 w payload: [p, kt, e]; kt<F -> e4m3(d*2^14), kt>=F -> e4m3(c*2^3)
    w8_d = nc.dram_tensor("w8", [128, 2 * F, E], f8, kind="ExternalInput")
    nbias_d = nc.dram_tensor("nbias", [128, E], f32, kind="ExternalInput")
    out_d = nc.dram_tensor("out", [128, nt * 2 * K], u32, kind="ExternalOutput")

    with tile.TileContext(nc) as tc:
        with (
            tc.tile_pool(name="const", bufs=1) as cpool,
            tc.tile_pool(name="xp", bufs=3) as xpool,
            tc.tile_pool(name="sc", bufs=4) as spool,
            tc.tile_pool(name="ps", bufs=4, space="PSUM") as ppool,
        ):
            # weights: chunked so early matmuls depend only on early chunks
            FC = F // 8
            c16_src = c16_d.ap().rearrange("(p f) e -> p f e", f=F)
            c16_c = []
            for c in range(8):
                fs = slice(c * FC, (c + 1) * FC)
                t = cpool.tile([128, FC, E], f16, tag=f"c16{c}")
                nc.scalar.dma_start(t[:], c16_src[:, fs, :])
                c16_c.append(t)
            w8_c = []
            for c in range(4):
                ks = slice(c * 16, (c + 1) * 16)
                t = cpool.tile([128, 16, E], f8, tag=f"w8{c}")
                nc.sync.dma_start(t[:], w8_d.ap()[:, ks, :])
                w8_c.append(t)
            nb = cpool.tile([128, E], f32, tag="nb")
            nc.sync.dma_start(nb[:], nbias_d.ap())
            obuf = cpool.tile([128, nt * 2 * K], u32, tag="obuf")

            FH = F // 4
            for it in range(nt):
                a_src = a16_d.ap()[it].rearrange("p (f t) -> p f t", f=F)
                x8_src = x8_d.ap()[it].rearrange("p (k t) -> p k t", k=2 * F)
                ach = []
                for c in range(4):
                    fs = slice(c * FH, (c + 1) * FH)
                    t = xpool.tile([128, FH, TCHUNK], f16, tag=f"a{c}")
                    nc.sync.dma_start(t[:], a_src[:, fs, :])
                    ach.append(t)
                x8ch = []
                for c in range(2):
                    ks = slice(c * F, (c + 1) * F)
                    t = xpool.tile([128, F, TCHUNK], f8, tag=f"x8{c}")
                    nc.scalar.dma_start(t[:], x8_src[:, ks, :])
                    x8ch.append(t)

                acc = ppool.tile([128, E], f32, tag="acc")
                for f in range(F):
                    nc.tensor.matmul(acc[:], ach[f // FH][:, f % FH, :],
                                     c16_c[f // FC][:, f % FC, :],
                                     start=(f == 0), stop=False)
                for fd in range(F):
                    kk = (2 * fd) % F
                    nc.tensor.matmul(acc[:],
                                     x8ch[fd // 16][:, kk:kk + 2, :],
                                     w8_c[fd // 8][:, (2 * fd) % 16:
                                                   (2 * fd) % 16 + 2, :],
                                     start=False, stop=(fd == F - 1),
                                     perf_mode=DR)

                scores = spool.tile([128, E], f32, tag="scores")
                nc.scalar.activation(scores[:], acc[:], Act.Sigmoid,
                                     scale=2.0 ** -14)

                negb = spool.tile([128, E], f32, tag="negb")
                nc.vector.tensor_tensor(negb[:], nb[:], scores[:], Alu.subtract)
                m8 = obuf[:, it * 2 * K + K: (it + 1) * 2 * K].bitcast(f32)
                nc.vector.max(m8, negb[:])
                nc.vector.max_index(obuf[:, it * 2 * K: it * 2 * K + K],
                                    m8, negb[:])

            nc.sync.dma_start(out_d.ap(), obuf[:])

    nc.compile()
    return nc


def host_prep(x, weight, e_score_correction_bias):
    """Split x/W into fp16 + scaled-fp8 payloads; pretile x per core."""
    x = np.asarray(x, dtype=np.float32)
    w = np.asarray(weight, dtype=np.float32)
    b = np.asarray(e_score_correction_bias, dtype=np.float32)

    a = x.astype(np.float16)
    bres = x - a.astype(np.float32)
    a16s = (a.astype(np.float32) * 128.0).astype(np.float16)  # exact *2^7
    a8 = a.astype(np.float32).astype(E4)
    b8s = (bres * 2.0 ** 11).astype(E4)

    wt = np.ascontiguousarray(w.T)     # [H, E]
    c = wt.astype(np.float16)
    d = wt - c.astype(np.float32)
    c16s = (c.astype(np.float32) * 128.0).astype(np.float16)  # exact *2^7
    d8s = (d * 2.0 ** 14).astype(E4)
    c8s = (c.astype(np.float32) * 8.0).astype(E4)

    def pretile2(arr):  # [TS, H] u16view -> [NT, 128, F*TCHUNK]
        arr = arr.reshape(NT, TCHUNK, 128, F).transpose(0, 2, 3, 1)
        return np.ascontiguousarray(arr).reshape(NT, 128, F * TCHUNK)

    w8 = np.empty((128, 2 * F, E), dtype=np.uint8)
    w8[:, :F, :] = d8s.view(np.uint8).reshape(128, F, E)
    w8[:, F:, :] = c8s.view(np.uint8).reshape(128, F, E)
    w8 = w8.view(E4)

    nbias = np.ascontiguousarray(np.broadcast_to(-b, (128, E)))

    in_maps = []
    for cid in range(N_CORES):
        sl = slice(cid * TS, (cid + 1) * TS)
        x8p = np.empty((NT, 128, 2 * F, TCHUNK), dtype=np.uint8)
        x8p[:, :, :F, :] = pretile2(a8[sl].view(np.uint8)).reshape(
            NT, 128, F, TCHUNK)
        x8p[:, :, F:, :] = pretile2(b8s[sl].view(np.uint8)).reshape(
            NT, 128, F, TCHUNK)
        in_maps.append({
            "a16": pretile2(a16s[sl].view(np.uint16)).view(np.float16),
            "x8": np.ascontiguousarray(x8p).reshape(
                NT, 128, 2 * F * TCHUNK).view(E4),
            "c16": c16s,
            "w8": w8,
            "nbias": nbias,
        })
    return in_maps


def unpack(out_cores, b):
    """list of [128, NT*16] u32 -> (inds int32 [T, 8], sel float32 [T, 8])."""
    inds = np.empty((T, K), dtype=np.int32)
    sel = np.empty((T, K), dtype=np.float32)
    for c, o in enumerate(out_cores):
        o = o.reshape(128, NT, 2 * K).transpose(1, 0, 2)  # [it, p, 16]
        o = np.ascontiguousarray(o).reshape(TS, 2 * K)
        ii = o[:, :K].astype(np.int32)
        keys = o[:, K:].view(np.float32)
        s = (-keys - b[ii]).astype(np.float32)   # selected original scores
        sv = s / s.sum(axis=-1, keepdims=True) * np.float32(ROUTED_SCALING)
        inds[c * TS:(c + 1) * TS] = ii
        sel[c * TS:(c + 1) * TS] = sv
    return inds, sel


_NC_CACHE = {}


def _get_nc():
    if "nc" not in _NC_CACHE:
        _NC_CACHE["nc"] = build_nc()
    return _NC_CACHE["nc"]


def kernel(x, weight, e_score_correction_bias, _trace=False):
    b = np.asarray(e_score_correction_bias, dtype=np.float32)
    in_maps = host_prep(x, weight, e_score_correction_bias)
    nc = _get_nc()
    res = bass_utils.run_bass_kernel_spmd(
        nc, in_maps, list(range(N_CORES)), trace=_trace)
    inds, sel = unpack([res.results[c]["out"] for c in range(N_CORES)], b)
    if _trace:
        kernel.last_results = res
    return inds, sel
